# revision 5
# baseline (speedup 1.0000x reference)
"""Grouped MoE (top-2 of 8 experts, SwiGLU) on 8 Trainium2 NeuronCores.

Sharding: expert-parallel with real token dispatch. The gate (softmax +
top-2 + renormalize) is computed on host as part of the sharding step;
tokens are gathered per expert into fixed-capacity buffers (CAP = max
expert count rounded up to 128). Core c owns expert c and runs the three
SwiGLU GEMMs in bf16 over only its own ~T*K/E tokens, scales each output
row by that token's gate weight on-device, and writes its [CAP, D] f32
shard. The host scatter-adds the two expert contributions per token back
into the full [T, D] output. No collectives are needed: each token's two
expert rows live on different cores and are summed on host.
"""

import sys
import numpy as np

for _p in ("/opt/trn_rl_repo",):
    if _p not in sys.path:
        sys.path.insert(0, _p)

B, S, D, F, E, K = 2, 2048, 1024, 1024, 8, 2
T = B * S            # 4096 tokens
NCORES = 8
P = 128
DK = D // P          # 8 contraction chunks over D
FK = F // P          # 8 F tiles
MAXCH = 512          # max token chunk (PSUM bank limit: 512 f32/partition)

_cache = {}


def _build_nc(cap):
    from contextlib import ExitStack

    import concourse.mybir as mybir
    import concourse.tile as tile
    from concourse import bacc

    dt = mybir.dt
    AF = mybir.ActivationFunctionType
    ALU = mybir.AluOpType

    ntiles = cap // P
    # token chunks of up to 512 (4 tiles)
    chunks = []
    off = 0
    while off < cap:
        tch = min(MAXCH, cap - off)
        chunks.append((off, tch))
        off += tch

    nc = bacc.Bacc("TRN2", target_bir_lowering=False, debug=False,
                   num_devices=NCORES)

    xg = nc.dram_tensor("xg", [D, cap], dt.bfloat16, kind="ExternalInput").ap()
    gsc = nc.dram_tensor("gsc", [P, ntiles], dt.float32,
                         kind="ExternalInput").ap()
    w1t = nc.dram_tensor("w1t", [D, F], dt.bfloat16, kind="ExternalInput").ap()
    w3t = nc.dram_tensor("w3t", [D, F], dt.bfloat16, kind="ExternalInput").ap()
    w2t = nc.dram_tensor("w2t", [F, D], dt.bfloat16, kind="ExternalInput").ap()
    out = nc.dram_tensor("out", [cap, D], dt.float32, kind="ExternalOutput").ap()

    with tile.TileContext(nc) as tc, ExitStack() as ctx:
        const = ctx.enter_context(tc.tile_pool(name="const", bufs=1))
        xpool = ctx.enter_context(tc.tile_pool(name="xpool", bufs=1))
        spool = ctx.enter_context(tc.tile_pool(name="spool", bufs=2))
        hpool = ctx.enter_context(tc.tile_pool(name="hpool", bufs=2))
        ypool = ctx.enter_context(tc.tile_pool(name="ypool", bufs=3))

        abpsum = ctx.enter_context(tc.tile_pool(name="abpsum", bufs=2,
                                                space="PSUM"))
        ypsum = ctx.enter_context(tc.tile_pool(name="ypsum", bufs=3,
                                               space="PSUM"))

        # ---- resident loads, split across the two HWDGE queues (sync +
        # scalar) so descriptor issue overlaps.  Critical-path order: the
        # first chunk's psA phase needs only xg chunk 0 (sync queue) and w1
        # (scalar queue); w3/w2/later x chunks stream in behind them. ----
        gsc_sb = const.tile([P, ntiles], dt.float32, tag="gsc")
        nc.sync.dma_start(gsc_sb[:], gsc[:, :])

        xg_sb = []
        for k in range(DK):
            tx = xpool.tile([P, cap], dt.bfloat16, tag=f"xg_{k}")
            xg_sb.append(tx)
        t0, tch0 = chunks[0]
        for k in range(DK):
            nc.sync.dma_start(xg_sb[k][:, t0:t0 + tch0],
                              xg[k * P:(k + 1) * P, t0:t0 + tch0])

        w1_sb = []
        for k in range(DK):
            t1 = const.tile([P, F], dt.bfloat16, tag=f"w1_{k}")
            nc.scalar.dma_start(t1[:], w1t[k * P:(k + 1) * P, :])
            w1_sb.append(t1)
        w3_sb = []
        for k in range(DK):
            t3 = const.tile([P, F], dt.bfloat16, tag=f"w3_{k}")
            nc.sync.dma_start(t3[:], w3t[k * P:(k + 1) * P, :])
            w3_sb.append(t3)
        w2_sb = []
        for k in range(FK):
            t2 = const.tile([P, D], dt.bfloat16, tag=f"w2_{k}")
            nc.scalar.dma_start(t2[:], w2t[k * P:(k + 1) * P, :])
            w2_sb.append(t2)
        if cap > tch0:
            for k in range(DK):
                nc.sync.dma_start(xg_sb[k][:, tch0:cap],
                                  xg[k * P:(k + 1) * P, tch0:cap])

        # ---- per-chunk SwiGLU FFN: psA phase (w1), psB phase (w3) with the
        # silu/mult combine trailing, then the w2 stage ----
        for (tok, tch) in chunks:
            s_sb = []
            for f in range(FK):
                psA = abpsum.tile([P, tch], dt.float32, tag="psA")
                for k in range(DK):
                    nc.tensor.matmul(
                        psA[:], lhsT=w1_sb[k][:, f * P:(f + 1) * P],
                        rhs=xg_sb[k][:, tok:tok + tch],
                        start=(k == 0), stop=(k == DK - 1))
                ssb = spool.tile([P, tch], dt.bfloat16, tag=f"s{f}")
                nc.scalar.activation(ssb[:], psA[:], AF.Silu)
                s_sb.append(ssb)
            h_sb = []
            for f in range(FK):
                psB = abpsum.tile([P, tch], dt.float32, tag="psB")
                for k in range(DK):
                    nc.tensor.matmul(
                        psB[:], lhsT=w3_sb[k][:, f * P:(f + 1) * P],
                        rhs=xg_sb[k][:, tok:tok + tch],
                        start=(k == 0), stop=(k == DK - 1))
                hsb = hpool.tile([P, tch], dt.bfloat16, tag=f"h{f}")
                nc.vector.tensor_tensor(hsb[:], s_sb[f][:], psB[:],
                                        op=ALU.mult)
                h_sb.append(hsb)
            for m in range(tch // P):
                jj = tok // P + m
                ysb = ypool.tile([P, D], dt.float32, tag="ysb")
                for nhalf in range(2):
                    psY = ypsum.tile([P, 512], dt.float32, tag="psY")
                    for fk in range(FK):
                        nc.tensor.matmul(
                            psY[:],
                            lhsT=h_sb[fk][:, m * P:(m + 1) * P],
                            rhs=w2_sb[fk][:, nhalf * 512:(nhalf + 1) * 512],
                            start=(fk == 0), stop=(fk == FK - 1))
                    nc.scalar.activation(ysb[:, nhalf * 512:(nhalf + 1) * 512],
                                         psY[:], AF.Copy,
                                         scale=gsc_sb[:, jj:jj + 1])
                nc.sync.dma_start(out[jj * P:(jj + 1) * P, :], ysb[:])

    nc.compile()
    return nc


def _route(xf, gate_w):
    """Host gate: returns per-expert (token indices, renormalized weights)."""
    logits = xf.astype(np.float64) @ gate_w.astype(np.float64).T   # [T, E]
    order = np.argsort(-logits, axis=1, kind="stable")
    i1 = order[:, 0]
    i2 = order[:, 1]
    ar = np.arange(T)
    l1 = logits[ar, i1]
    l2 = logits[ar, i2]
    g1 = 1.0 / (1.0 + np.exp(l2 - l1))
    g2 = 1.0 - g1
    idx_e, scl_e = [], []
    for e in range(E):
        m1 = i1 == e
        m2 = i2 == e
        ids = np.concatenate([np.nonzero(m1)[0], np.nonzero(m2)[0]])
        sc = np.concatenate([g1[m1], g2[m2]])
        idx_e.append(ids)
        scl_e.append(sc.astype(np.float32))
    return idx_e, scl_e


def prepare(x, gate_w, w1, w3, w2):
    """Host routing + sharding: returns (nc, in_maps, idx_e)."""
    import ml_dtypes

    xf = np.ascontiguousarray(x.reshape(T, D).astype(np.float32))
    xTb = np.ascontiguousarray(xf.T).astype(ml_dtypes.bfloat16)   # [D, T]

    idx_e, scl_e = _route(xf, gate_w)
    maxcnt = max(len(i) for i in idx_e)
    cap = ((maxcnt + P - 1) // P) * P
    ntiles = cap // P

    if cap not in _cache:
        _cache[cap] = _build_nc(cap)
    nc = _cache[cap]

    in_maps = []
    for c in range(NCORES):
        ids = idx_e[c]
        cnt = len(ids)
        xg_c = np.zeros((D, cap), dtype=ml_dtypes.bfloat16)
        xg_c[:, :cnt] = xTb[:, ids]
        sc = np.zeros(cap, dtype=np.float32)
        sc[:cnt] = scl_e[c]
        gsc_c = np.ascontiguousarray(sc.reshape(ntiles, P).T)     # [P, ntiles]
        in_maps.append({
            "xg": xg_c,
            "gsc": gsc_c,
            "w1t": np.ascontiguousarray(w1[c].T).astype(ml_dtypes.bfloat16),
            "w3t": np.ascontiguousarray(w3[c].T).astype(ml_dtypes.bfloat16),
            "w2t": np.ascontiguousarray(w2[c].T).astype(ml_dtypes.bfloat16),
        })
    return nc, in_maps, idx_e


def _combine(res, idx_e):
    outf = np.zeros((T, D), dtype=np.float32)
    for c in range(NCORES):
        cnt = len(idx_e[c])
        outf[idx_e[c]] += res.results[c]["out"][:cnt]
    return outf.reshape(B, S, D)


def kernel(x, gate_w, w1, w3, w2):
    from concourse.bass_utils import run_bass_kernel_spmd

    nc, in_maps, idx_e = prepare(x, gate_w, w1, w3, w2)
    res = run_bass_kernel_spmd(nc, in_maps, list(range(NCORES)))
    return _combine(res, idx_e)


# revision 7
# speedup vs baseline: 1.0412x; 1.0412x over previous
"""Grouped MoE (top-2 of 8 experts, SwiGLU) on 8 Trainium2 NeuronCores.

Sharding: expert-parallel with real token dispatch. The gate (softmax +
top-2 + renormalize) is computed on host as part of the sharding step;
tokens are gathered per expert into fixed-capacity buffers (CAP = max
expert count rounded up to 128). Core c owns expert c and runs the three
SwiGLU GEMMs in bf16 over only its own ~T*K/E tokens, scales each output
row by that token's gate weight on-device, and writes its [CAP, D] f32
shard. The host scatter-adds the two expert contributions per token back
into the full [T, D] output. No collectives are needed: each token's two
expert rows live on different cores and are summed on host.
"""

import sys
import numpy as np

for _p in ("/opt/trn_rl_repo",):
    if _p not in sys.path:
        sys.path.insert(0, _p)

B, S, D, F, E, K = 2, 2048, 1024, 1024, 8, 2
T = B * S            # 4096 tokens
NCORES = 8
P = 128
DK = D // P          # 8 contraction chunks over D
FK = F // P          # 8 F tiles
MAXCH = 512          # max token chunk (PSUM bank limit: 512 f32/partition)

_cache = {}


def _build_nc(cap):
    from contextlib import ExitStack

    import concourse.mybir as mybir
    import concourse.tile as tile
    from concourse import bacc

    dt = mybir.dt
    AF = mybir.ActivationFunctionType
    ALU = mybir.AluOpType

    ntiles = cap // P
    # token chunks of up to 512 (4 tiles)
    chunks = []
    off = 0
    while off < cap:
        tch = min(MAXCH, cap - off)
        chunks.append((off, tch))
        off += tch

    nc = bacc.Bacc("TRN2", target_bir_lowering=False, debug=False,
                   num_devices=NCORES)

    xg = nc.dram_tensor("xg", [D, cap], dt.bfloat16, kind="ExternalInput").ap()
    gsc = nc.dram_tensor("gsc", [P, ntiles], dt.float32,
                         kind="ExternalInput").ap()
    w1t = nc.dram_tensor("w1t", [D, F], dt.bfloat16, kind="ExternalInput").ap()
    w3t = nc.dram_tensor("w3t", [D, F], dt.bfloat16, kind="ExternalInput").ap()
    w2t = nc.dram_tensor("w2t", [F, D], dt.bfloat16, kind="ExternalInput").ap()
    out = nc.dram_tensor("out", [cap, D], dt.float32, kind="ExternalOutput").ap()

    with tile.TileContext(nc) as tc, ExitStack() as ctx:
        const = ctx.enter_context(tc.tile_pool(name="const", bufs=1))
        xpool = ctx.enter_context(tc.tile_pool(name="xpool", bufs=1))
        spool = ctx.enter_context(tc.tile_pool(name="spool", bufs=2))
        hpool = ctx.enter_context(tc.tile_pool(name="hpool", bufs=2))
        ypool = ctx.enter_context(tc.tile_pool(name="ypool", bufs=3))

        abpsum = ctx.enter_context(tc.tile_pool(name="abpsum", bufs=2,
                                                space="PSUM"))
        ypsum = ctx.enter_context(tc.tile_pool(name="ypsum", bufs=2,
                                               space="PSUM"))

        # ---- resident loads, all on the sync HWDGE ring (the scalar engine
        # must stay DMA-free: dma_start instructions would delay silu/copy in
        # scalar program order and stall PSUM recycling).  Order: gate
        # scales, x chunk 0, w1, w3 (first chunk's stage-A inputs), then w2
        # and the x tail as single multi-dim-AP transfers. ----
        gsc_sb = const.tile([P, ntiles], dt.float32, tag="gsc")
        nc.sync.dma_start(gsc_sb[:], gsc[:, :])

        xall = xpool.tile([P, DK * cap], dt.bfloat16, tag="xall")
        xg_sb = [xall[:, k * cap:(k + 1) * cap] for k in range(DK)]
        t0, tch0 = chunks[0]
        for k in range(DK):
            nc.sync.dma_start(xg_sb[k][:, t0:t0 + tch0],
                              xg[k * P:(k + 1) * P, t0:t0 + tch0])

        w1_sb = []
        for k in range(DK):
            t1 = const.tile([P, F], dt.bfloat16, tag=f"w1_{k}")
            nc.sync.dma_start(t1[:], w1t[k * P:(k + 1) * P, :])
            w1_sb.append(t1)
        w3_sb = []
        for k in range(DK):
            t3 = const.tile([P, F], dt.bfloat16, tag=f"w3_{k}")
            nc.sync.dma_start(t3[:], w3t[k * P:(k + 1) * P, :])
            w3_sb.append(t3)

        w2all = const.tile([P, FK * D], dt.bfloat16, tag="w2all")
        nc.sync.dma_start(
            w2all[:].rearrange("p (k d) -> p k d", k=FK),
            w2t.rearrange("(k p) d -> p k d", p=P))
        w2_sb = [w2all[:, k * D:(k + 1) * D] for k in range(FK)]

        if cap > tch0:
            nc.sync.dma_start(
                xall[:].rearrange("p (k t) -> p k t", k=DK)[:, :, tch0:cap],
                xg.rearrange("(k p) t -> p k t", p=P)[:, :, tch0:cap])

        # ---- per-chunk SwiGLU FFN ----
        for (tok, tch) in chunks:
            h_sb = []
            for f in range(FK):
                psA = abpsum.tile([P, tch], dt.float32, tag="psA")
                for k in range(DK):
                    nc.tensor.matmul(
                        psA[:], lhsT=w1_sb[k][:, f * P:(f + 1) * P],
                        rhs=xg_sb[k][:, tok:tok + tch],
                        start=(k == 0), stop=(k == DK - 1))
                psB = abpsum.tile([P, tch], dt.float32, tag="psB")
                for k in range(DK):
                    nc.tensor.matmul(
                        psB[:], lhsT=w3_sb[k][:, f * P:(f + 1) * P],
                        rhs=xg_sb[k][:, tok:tok + tch],
                        start=(k == 0), stop=(k == DK - 1))
                ssb = spool.tile([P, tch], dt.bfloat16, tag="ssb")
                nc.scalar.activation(ssb[:], psA[:], AF.Silu)
                hsb = hpool.tile([P, tch], dt.bfloat16, tag=f"h{f}")
                nc.vector.tensor_tensor(hsb[:], ssb[:], psB[:], op=ALU.mult)
                h_sb.append(hsb)
            for m in range(tch // P):
                jj = tok // P + m
                psY = ypsum.tile([P, D], dt.float32, tag="psY")
                for nhalf in range(2):
                    for fk in range(FK):
                        nc.tensor.matmul(
                            psY[:, nhalf * 512:(nhalf + 1) * 512],
                            lhsT=h_sb[fk][:, m * P:(m + 1) * P],
                            rhs=w2_sb[fk][:, nhalf * 512:(nhalf + 1) * 512],
                            start=(fk == 0), stop=(fk == FK - 1))
                ysb = ypool.tile([P, D], dt.float32, tag="ysb")
                nc.scalar.activation(ysb[:], psY[:], AF.Copy,
                                     scale=gsc_sb[:, jj:jj + 1])
                nc.sync.dma_start(out[jj * P:(jj + 1) * P, :], ysb[:])

    nc.compile()
    return nc


def _route(xf, gate_w):
    """Host gate: returns per-expert (token indices, renormalized weights)."""
    logits = xf.astype(np.float64) @ gate_w.astype(np.float64).T   # [T, E]
    order = np.argsort(-logits, axis=1, kind="stable")
    i1 = order[:, 0]
    i2 = order[:, 1]
    ar = np.arange(T)
    l1 = logits[ar, i1]
    l2 = logits[ar, i2]
    g1 = 1.0 / (1.0 + np.exp(l2 - l1))
    g2 = 1.0 - g1
    idx_e, scl_e = [], []
    for e in range(E):
        m1 = i1 == e
        m2 = i2 == e
        ids = np.concatenate([np.nonzero(m1)[0], np.nonzero(m2)[0]])
        sc = np.concatenate([g1[m1], g2[m2]])
        idx_e.append(ids)
        scl_e.append(sc.astype(np.float32))
    return idx_e, scl_e


def prepare(x, gate_w, w1, w3, w2):
    """Host routing + sharding: returns (nc, in_maps, idx_e)."""
    import ml_dtypes

    xf = np.ascontiguousarray(x.reshape(T, D).astype(np.float32))
    xTb = np.ascontiguousarray(xf.T).astype(ml_dtypes.bfloat16)   # [D, T]

    idx_e, scl_e = _route(xf, gate_w)
    maxcnt = max(len(i) for i in idx_e)
    cap = ((maxcnt + P - 1) // P) * P
    ntiles = cap // P

    if cap not in _cache:
        _cache[cap] = _build_nc(cap)
    nc = _cache[cap]

    in_maps = []
    for c in range(NCORES):
        ids = idx_e[c]
        cnt = len(ids)
        xg_c = np.zeros((D, cap), dtype=ml_dtypes.bfloat16)
        xg_c[:, :cnt] = xTb[:, ids]
        sc = np.zeros(cap, dtype=np.float32)
        sc[:cnt] = scl_e[c]
        gsc_c = np.ascontiguousarray(sc.reshape(ntiles, P).T)     # [P, ntiles]
        in_maps.append({
            "xg": xg_c,
            "gsc": gsc_c,
            "w1t": np.ascontiguousarray(w1[c].T).astype(ml_dtypes.bfloat16),
            "w3t": np.ascontiguousarray(w3[c].T).astype(ml_dtypes.bfloat16),
            "w2t": np.ascontiguousarray(w2[c].T).astype(ml_dtypes.bfloat16),
        })
    return nc, in_maps, idx_e


def _combine(res, idx_e):
    outf = np.zeros((T, D), dtype=np.float32)
    for c in range(NCORES):
        cnt = len(idx_e[c])
        outf[idx_e[c]] += res.results[c]["out"][:cnt]
    return outf.reshape(B, S, D)


def kernel(x, gate_w, w1, w3, w2):
    from concourse.bass_utils import run_bass_kernel_spmd

    nc, in_maps, idx_e = prepare(x, gate_w, w1, w3, w2)
    res = run_bass_kernel_spmd(nc, in_maps, list(range(NCORES)))
    return _combine(res, idx_e)


# revision 12
# speedup vs baseline: 1.0454x; 1.0040x over previous
"""Grouped MoE (top-2 of 8 experts, SwiGLU) on 8 Trainium2 NeuronCores.

Sharding: expert-parallel with real token dispatch. The gate (softmax +
top-2 + renormalize) is computed on host as part of the sharding step;
tokens are gathered per expert into fixed-capacity buffers (CAP = max
expert count rounded up to 128). Core c owns expert c and runs the three
SwiGLU GEMMs in bf16 over only its own ~T*K/E tokens, scales each output
row by that token's gate weight on-device, and writes its [CAP, D] f32
shard. The host scatter-adds the two expert contributions per token back
into the full [T, D] output. No collectives are needed: each token's two
expert rows live on different cores and are summed on host.
"""

import sys
import numpy as np

for _p in ("/opt/trn_rl_repo",):
    if _p not in sys.path:
        sys.path.insert(0, _p)

B, S, D, F, E, K = 2, 2048, 1024, 1024, 8, 2
T = B * S            # 4096 tokens
NCORES = 8
P = 128
DK = D // P          # 8 contraction chunks over D
FK = F // P          # 8 F tiles
MAXCH = 512          # max token chunk (PSUM bank limit: 512 f32/partition)

_cache = {}


def _build_nc(cap):
    from contextlib import ExitStack

    import concourse.mybir as mybir
    import concourse.tile as tile
    from concourse import bacc

    dt = mybir.dt
    AF = mybir.ActivationFunctionType
    ALU = mybir.AluOpType

    ntiles = (cap + P - 1) // P
    # token chunks of up to 512 (4 tiles)
    chunks = []
    off = 0
    while off < cap:
        tch = min(MAXCH, cap - off)
        chunks.append((off, tch))
        off += tch

    nc = bacc.Bacc("TRN2", target_bir_lowering=False, debug=False,
                   num_devices=NCORES)

    xg = nc.dram_tensor("xg", [D, cap], dt.bfloat16, kind="ExternalInput").ap()
    gsc = nc.dram_tensor("gsc", [P, ntiles], dt.float32,
                         kind="ExternalInput").ap()
    w1t = nc.dram_tensor("w1t", [D, F], dt.bfloat16, kind="ExternalInput").ap()
    w3t = nc.dram_tensor("w3t", [D, F], dt.bfloat16, kind="ExternalInput").ap()
    w2t = nc.dram_tensor("w2t", [F, D], dt.bfloat16, kind="ExternalInput").ap()
    out = nc.dram_tensor("out", [cap, D], dt.float32, kind="ExternalOutput").ap()

    with tile.TileContext(nc) as tc, ExitStack() as ctx:
        const = ctx.enter_context(tc.tile_pool(name="const", bufs=1))
        xpool = ctx.enter_context(tc.tile_pool(name="xpool", bufs=1))
        spool = ctx.enter_context(tc.tile_pool(name="spool", bufs=2))
        hpool = ctx.enter_context(tc.tile_pool(name="hpool", bufs=2))
        ypool = ctx.enter_context(tc.tile_pool(name="ypool", bufs=3))

        abpsum = ctx.enter_context(tc.tile_pool(name="abpsum", bufs=2,
                                                space="PSUM"))
        ypsum = ctx.enter_context(tc.tile_pool(name="ypsum", bufs=2,
                                               space="PSUM"))

        # ---- resident loads.  Sync ring carries the stage-A critical path
        # (x chunk 0 as one multi-AP DMA, then w1 per-k) plus the x tail and
        # output stores; the scalar ring carries w3/w2 — 9 descriptor issues
        # that finish well before the first silu needs the scalar engine. ----
        xall = xpool.tile([P, DK * cap], dt.bfloat16, tag="xall")
        xg_sb = [xall[:, k * cap:(k + 1) * cap] for k in range(DK)]
        t0, tch0 = chunks[0]
        nc.sync.dma_start(
            xall[:].rearrange("p (k t) -> p k t", k=DK)[:, :, t0:tch0],
            xg.rearrange("(k p) t -> p k t", p=P)[:, :, t0:tch0])

        w1_sb = []
        for k in range(DK):
            t1 = const.tile([P, F], dt.bfloat16, tag=f"w1_{k}")
            nc.sync.dma_start(t1[:], w1t[k * P:(k + 1) * P, :])
            w1_sb.append(t1)
        w3_sb = []
        for k in range(DK):
            t3 = const.tile([P, F], dt.bfloat16, tag=f"w3_{k}")
            nc.scalar.dma_start(t3[:], w3t[k * P:(k + 1) * P, :])
            w3_sb.append(t3)

        gsc_sb = const.tile([P, ntiles], dt.float32, tag="gsc")
        nc.sync.dma_start(gsc_sb[:], gsc[:, :])

        w2all = const.tile([P, FK * D], dt.bfloat16, tag="w2all")
        nc.scalar.dma_start(
            w2all[:].rearrange("p (k d) -> p k d", k=FK),
            w2t.rearrange("(k p) d -> p k d", p=P))
        w2_sb = [w2all[:, k * D:(k + 1) * D] for k in range(FK)]

        if cap > tch0:
            nc.sync.dma_start(
                xall[:].rearrange("p (k t) -> p k t", k=DK)[:, :, tch0:cap],
                xg.rearrange("(k p) t -> p k t", p=P)[:, :, tch0:cap])

        # ---- per-chunk SwiGLU FFN ----
        for (tok, tch) in chunks:
            h_sb = []
            for f in range(FK):
                psA = abpsum.tile([P, tch], dt.float32, tag="psA")
                for k in range(DK):
                    nc.tensor.matmul(
                        psA[:], lhsT=w1_sb[k][:, f * P:(f + 1) * P],
                        rhs=xg_sb[k][:, tok:tok + tch],
                        start=(k == 0), stop=(k == DK - 1))
                psB = abpsum.tile([P, tch], dt.float32, tag="psB")
                for k in range(DK):
                    nc.tensor.matmul(
                        psB[:], lhsT=w3_sb[k][:, f * P:(f + 1) * P],
                        rhs=xg_sb[k][:, tok:tok + tch],
                        start=(k == 0), stop=(k == DK - 1))
                ssb = spool.tile([P, tch], dt.bfloat16, tag="ssb")
                nc.scalar.activation(ssb[:], psA[:], AF.Silu)
                hsb = hpool.tile([P, tch], dt.bfloat16, tag=f"h{f}")
                nc.vector.tensor_tensor(hsb[:], ssb[:], psB[:], op=ALU.mult)
                h_sb.append(hsb)
            for m in range((tch + P - 1) // P):
                jj = tok // P + m
                pm = min(P, tch - m * P)
                last = (tok + m * P + pm == cap)
                psY = ypsum.tile([P, D], dt.float32, tag="psY")
                for nhalf in range(2):
                    for fk in range(FK):
                        nc.tensor.matmul(
                            psY[:pm, nhalf * 512:(nhalf + 1) * 512],
                            lhsT=h_sb[fk][:, m * P:m * P + pm],
                            rhs=w2_sb[fk][:, nhalf * 512:(nhalf + 1) * 512],
                            start=(fk == 0), stop=(fk == FK - 1))
                ysb = ypool.tile([P, D], dt.float32, tag="ysb")
                if last:
                    # split the final copy+store so the first half's DMA
                    # overlaps the second half's copy (shorter kernel tail)
                    for nhalf in range(2):
                        nc.scalar.activation(
                            ysb[:pm, nhalf * 512:(nhalf + 1) * 512],
                            psY[:pm, nhalf * 512:(nhalf + 1) * 512],
                            AF.Copy, scale=gsc_sb[:pm, jj:jj + 1])
                        nc.sync.dma_start(
                            out[jj * P:jj * P + pm,
                                nhalf * 512:(nhalf + 1) * 512],
                            ysb[:pm, nhalf * 512:(nhalf + 1) * 512])
                else:
                    nc.scalar.activation(ysb[:pm, :], psY[:pm, :], AF.Copy,
                                         scale=gsc_sb[:pm, jj:jj + 1])
                    nc.sync.dma_start(out[jj * P:jj * P + pm, :], ysb[:pm, :])

    nc.compile()
    return nc


def _route(xf, gate_w):
    """Host gate: returns per-expert (token indices, renormalized weights)."""
    logits = xf.astype(np.float64) @ gate_w.astype(np.float64).T   # [T, E]
    order = np.argsort(-logits, axis=1, kind="stable")
    i1 = order[:, 0]
    i2 = order[:, 1]
    ar = np.arange(T)
    l1 = logits[ar, i1]
    l2 = logits[ar, i2]
    g1 = 1.0 / (1.0 + np.exp(l2 - l1))
    g2 = 1.0 - g1
    idx_e, scl_e = [], []
    for e in range(E):
        m1 = i1 == e
        m2 = i2 == e
        ids = np.concatenate([np.nonzero(m1)[0], np.nonzero(m2)[0]])
        sc = np.concatenate([g1[m1], g2[m2]])
        idx_e.append(ids)
        scl_e.append(sc.astype(np.float32))
    return idx_e, scl_e


def prepare(x, gate_w, w1, w3, w2):
    """Host routing + sharding: returns (nc, in_maps, idx_e)."""
    import ml_dtypes

    xf = np.ascontiguousarray(x.reshape(T, D).astype(np.float32))
    xTb = np.ascontiguousarray(xf.T).astype(ml_dtypes.bfloat16)   # [D, T]

    idx_e, scl_e = _route(xf, gate_w)
    maxcnt = max(len(i) for i in idx_e)
    cap = ((maxcnt + 63) // 64) * 64
    ntiles = (cap + P - 1) // P

    if cap not in _cache:
        _cache[cap] = _build_nc(cap)
    nc = _cache[cap]

    in_maps = []
    for c in range(NCORES):
        ids = idx_e[c]
        cnt = len(ids)
        xg_c = np.zeros((D, cap), dtype=ml_dtypes.bfloat16)
        xg_c[:, :cnt] = xTb[:, ids]
        sc = np.zeros(ntiles * P, dtype=np.float32)
        sc[:cnt] = scl_e[c]
        gsc_c = np.ascontiguousarray(sc.reshape(ntiles, P).T)     # [P, ntiles]
        in_maps.append({
            "xg": xg_c,
            "gsc": gsc_c,
            "w1t": np.ascontiguousarray(w1[c].T).astype(ml_dtypes.bfloat16),
            "w3t": np.ascontiguousarray(w3[c].T).astype(ml_dtypes.bfloat16),
            "w2t": np.ascontiguousarray(w2[c].T).astype(ml_dtypes.bfloat16),
        })
    return nc, in_maps, idx_e


def _combine(res, idx_e):
    outf = np.zeros((T, D), dtype=np.float32)
    for c in range(NCORES):
        cnt = len(idx_e[c])
        outf[idx_e[c]] += res.results[c]["out"][:cnt]
    return outf.reshape(B, S, D)


def kernel(x, gate_w, w1, w3, w2):
    from concourse.bass_utils import run_bass_kernel_spmd

    nc, in_maps, idx_e = prepare(x, gate_w, w1, w3, w2)
    res = run_bass_kernel_spmd(nc, in_maps, list(range(NCORES)))
    return _combine(res, idx_e)


# revision 14
# speedup vs baseline: 1.1049x; 1.0570x over previous
"""Grouped MoE (top-2 of 8 experts, SwiGLU) on 8 Trainium2 NeuronCores.

Sharding: expert-parallel with real token dispatch. The gate (softmax +
top-2 + renormalize) is computed on host as part of the sharding step;
tokens are gathered per expert into fixed-capacity buffers (CAP = max
expert count rounded up to 128). Core c owns expert c and runs the three
SwiGLU GEMMs in bf16 over only its own ~T*K/E tokens, scales each output
row by that token's gate weight on-device, and writes its [CAP, D] f32
shard. The host scatter-adds the two expert contributions per token back
into the full [T, D] output. No collectives are needed: each token's two
expert rows live on different cores and are summed on host.
"""

import sys
import numpy as np

for _p in ("/opt/trn_rl_repo",):
    if _p not in sys.path:
        sys.path.insert(0, _p)

B, S, D, F, E, K = 2, 2048, 1024, 1024, 8, 2
T = B * S            # 4096 tokens
NCORES = 8
P = 128
DK = D // P          # 8 contraction chunks over D
FK = F // P          # 8 F tiles
MAXCH = 512          # max token chunk (PSUM bank limit: 512 f32/partition)

_cache = {}


def _build_nc(cap):
    from contextlib import ExitStack

    import concourse.mybir as mybir
    import concourse.tile as tile
    from concourse import bacc

    dt = mybir.dt
    AF = mybir.ActivationFunctionType
    ALU = mybir.AluOpType

    ntiles = (cap + P - 1) // P
    # token chunks of up to 512 f32 (PSUM bank limit).  All chunks are
    # multiples of 128 except possibly the last; avoid chunks < 128 (the
    # ~60-cycle NX dispatch floor makes n=64 matmuls cost nearly as much as
    # n=192 ones) by carving a 192 remainder when cap % 128 == 64.
    sizes = []
    rem = cap
    while rem > 512:
        if rem % 128 == 64 and rem <= 512 + 192:
            break
        sizes.append(512)
        rem -= 512
    if rem % 128 == 64 and rem > 192:
        sizes += [rem - 192, 192]
    elif rem:
        sizes.append(rem)
    chunks = []
    off = 0
    for tch in sizes:
        chunks.append((off, tch))
        off += tch

    nc = bacc.Bacc("TRN2", target_bir_lowering=False, debug=False,
                   num_devices=NCORES)

    xg = nc.dram_tensor("xg", [D, cap], dt.bfloat16, kind="ExternalInput").ap()
    gsc = nc.dram_tensor("gsc", [P, ntiles], dt.float32,
                         kind="ExternalInput").ap()
    w1t = nc.dram_tensor("w1t", [D, F], dt.bfloat16, kind="ExternalInput").ap()
    w3t = nc.dram_tensor("w3t", [D, F], dt.bfloat16, kind="ExternalInput").ap()
    w2t = nc.dram_tensor("w2t", [F, D], dt.bfloat16, kind="ExternalInput").ap()
    out = nc.dram_tensor("out", [cap, D], dt.float32, kind="ExternalOutput").ap()

    with tile.TileContext(nc) as tc, ExitStack() as ctx:
        const = ctx.enter_context(tc.tile_pool(name="const", bufs=1))
        xpool = ctx.enter_context(tc.tile_pool(name="xpool", bufs=1))
        spool = ctx.enter_context(tc.tile_pool(name="spool", bufs=2))
        hpool = ctx.enter_context(tc.tile_pool(name="hpool", bufs=2))
        ypool = ctx.enter_context(tc.tile_pool(name="ypool", bufs=3))

        abpsum = ctx.enter_context(tc.tile_pool(name="abpsum", bufs=2,
                                                space="PSUM"))
        ypsum = ctx.enter_context(tc.tile_pool(name="ypsum", bufs=2,
                                               space="PSUM"))

        # ---- resident loads.  Sync ring carries the stage-A critical path
        # (x chunk 0 as one multi-AP DMA, then w1 per-k) plus the x tail and
        # output stores; the scalar ring carries w3/w2 — 9 descriptor issues
        # that finish well before the first silu needs the scalar engine. ----
        xall = xpool.tile([P, DK * cap], dt.bfloat16, tag="xall")
        xg_sb = [xall[:, k * cap:(k + 1) * cap] for k in range(DK)]
        t0, tch0 = chunks[0]
        nc.sync.dma_start(
            xall[:].rearrange("p (k t) -> p k t", k=DK)[:, :, t0:tch0],
            xg.rearrange("(k p) t -> p k t", p=P)[:, :, t0:tch0])

        w1_sb = []
        for k in range(DK):
            t1 = const.tile([P, F], dt.bfloat16, tag=f"w1_{k}")
            nc.sync.dma_start(t1[:], w1t[k * P:(k + 1) * P, :])
            w1_sb.append(t1)
        w3_sb = []
        for k in range(DK):
            t3 = const.tile([P, F], dt.bfloat16, tag=f"w3_{k}")
            nc.sync.dma_start(t3[:], w3t[k * P:(k + 1) * P, :])
            w3_sb.append(t3)

        gsc_sb = const.tile([P, ntiles], dt.float32, tag="gsc")
        nc.sync.dma_start(gsc_sb[:], gsc[:, :])

        w2all = const.tile([P, FK * D], dt.bfloat16, tag="w2all")
        nc.sync.dma_start(
            w2all[:].rearrange("p (k d) -> p k d", k=FK),
            w2t.rearrange("(k p) d -> p k d", p=P))
        w2_sb = [w2all[:, k * D:(k + 1) * D] for k in range(FK)]

        if cap > tch0:
            nc.sync.dma_start(
                xall[:].rearrange("p (k t) -> p k t", k=DK)[:, :, tch0:cap],
                xg.rearrange("(k p) t -> p k t", p=P)[:, :, tch0:cap])

        # ---- PE warm-up: dummy matmuls while the weight DMAs are in flight
        # keep the tensor engine's activity window full so HAM reaches the
        # 2.4 GHz p-state before the real stream begins ----
        wrm = spool.tile([P, 512], dt.bfloat16, tag="wrm")
        nc.vector.memset(wrm[:], 0.5)
        psW = abpsum.tile([P, 512], dt.float32, tag="psA")
        for _ in range(10):
            nc.tensor.matmul(psW[:], lhsT=wrm[:, 0:P], rhs=wrm[:],
                             start=True, stop=True)

        # ---- per-chunk SwiGLU FFN ----
        for (tok, tch) in chunks:
            h_sb = []
            for f in range(FK):
                psA = abpsum.tile([P, tch], dt.float32, tag="psA")
                for k in range(DK):
                    nc.tensor.matmul(
                        psA[:], lhsT=w1_sb[k][:, f * P:(f + 1) * P],
                        rhs=xg_sb[k][:, tok:tok + tch],
                        start=(k == 0), stop=(k == DK - 1))
                psB = abpsum.tile([P, tch], dt.float32, tag="psB")
                for k in range(DK):
                    nc.tensor.matmul(
                        psB[:], lhsT=w3_sb[k][:, f * P:(f + 1) * P],
                        rhs=xg_sb[k][:, tok:tok + tch],
                        start=(k == 0), stop=(k == DK - 1))
                ssb = spool.tile([P, tch], dt.bfloat16, tag="ssb")
                nc.scalar.activation(ssb[:], psA[:], AF.Silu)
                hsb = hpool.tile([P, tch], dt.bfloat16, tag=f"h{f}")
                nc.vector.tensor_tensor(hsb[:], ssb[:], psB[:], op=ALU.mult)
                h_sb.append(hsb)
            for m in range((tch + P - 1) // P):
                jj = tok // P + m
                pm = min(P, tch - m * P)
                last = (tok + m * P + pm == cap)
                psY = ypsum.tile([P, D], dt.float32, tag="psY")
                for nhalf in range(2):
                    for fk in range(FK):
                        nc.tensor.matmul(
                            psY[:pm, nhalf * 512:(nhalf + 1) * 512],
                            lhsT=h_sb[fk][:, m * P:m * P + pm],
                            rhs=w2_sb[fk][:, nhalf * 512:(nhalf + 1) * 512],
                            start=(fk == 0), stop=(fk == FK - 1))
                ysb = ypool.tile([P, D], dt.float32, tag="ysb")
                if last:
                    # split the final copy+store so the first half's DMA
                    # overlaps the second half's copy (shorter kernel tail)
                    for nhalf in range(2):
                        nc.scalar.activation(
                            ysb[:pm, nhalf * 512:(nhalf + 1) * 512],
                            psY[:pm, nhalf * 512:(nhalf + 1) * 512],
                            AF.Copy, scale=gsc_sb[:pm, jj:jj + 1])
                        nc.sync.dma_start(
                            out[jj * P:jj * P + pm,
                                nhalf * 512:(nhalf + 1) * 512],
                            ysb[:pm, nhalf * 512:(nhalf + 1) * 512])
                else:
                    nc.scalar.activation(ysb[:pm, :], psY[:pm, :], AF.Copy,
                                         scale=gsc_sb[:pm, jj:jj + 1])
                    nc.sync.dma_start(out[jj * P:jj * P + pm, :], ysb[:pm, :])

    nc.compile()
    return nc


def _route(xf, gate_w):
    """Host gate: returns per-expert (token indices, renormalized weights)."""
    logits = xf.astype(np.float64) @ gate_w.astype(np.float64).T   # [T, E]
    order = np.argsort(-logits, axis=1, kind="stable")
    i1 = order[:, 0]
    i2 = order[:, 1]
    ar = np.arange(T)
    l1 = logits[ar, i1]
    l2 = logits[ar, i2]
    g1 = 1.0 / (1.0 + np.exp(l2 - l1))
    g2 = 1.0 - g1
    idx_e, scl_e = [], []
    for e in range(E):
        m1 = i1 == e
        m2 = i2 == e
        ids = np.concatenate([np.nonzero(m1)[0], np.nonzero(m2)[0]])
        sc = np.concatenate([g1[m1], g2[m2]])
        idx_e.append(ids)
        scl_e.append(sc.astype(np.float32))
    return idx_e, scl_e


def prepare(x, gate_w, w1, w3, w2):
    """Host routing + sharding: returns (nc, in_maps, idx_e)."""
    import ml_dtypes

    xf = np.ascontiguousarray(x.reshape(T, D).astype(np.float32))
    xTb = np.ascontiguousarray(xf.T).astype(ml_dtypes.bfloat16)   # [D, T]

    idx_e, scl_e = _route(xf, gate_w)
    maxcnt = max(len(i) for i in idx_e)
    cap = ((maxcnt + 63) // 64) * 64
    ntiles = (cap + P - 1) // P

    if cap not in _cache:
        _cache[cap] = _build_nc(cap)
    nc = _cache[cap]

    in_maps = []
    for c in range(NCORES):
        ids = idx_e[c]
        cnt = len(ids)
        xg_c = np.zeros((D, cap), dtype=ml_dtypes.bfloat16)
        xg_c[:, :cnt] = xTb[:, ids]
        sc = np.zeros(ntiles * P, dtype=np.float32)
        sc[:cnt] = scl_e[c]
        gsc_c = np.ascontiguousarray(sc.reshape(ntiles, P).T)     # [P, ntiles]
        in_maps.append({
            "xg": xg_c,
            "gsc": gsc_c,
            "w1t": np.ascontiguousarray(w1[c].T).astype(ml_dtypes.bfloat16),
            "w3t": np.ascontiguousarray(w3[c].T).astype(ml_dtypes.bfloat16),
            "w2t": np.ascontiguousarray(w2[c].T).astype(ml_dtypes.bfloat16),
        })
    return nc, in_maps, idx_e


def _combine(res, idx_e):
    outf = np.zeros((T, D), dtype=np.float32)
    for c in range(NCORES):
        cnt = len(idx_e[c])
        outf[idx_e[c]] += res.results[c]["out"][:cnt]
    return outf.reshape(B, S, D)


def kernel(x, gate_w, w1, w3, w2):
    from concourse.bass_utils import run_bass_kernel_spmd

    nc, in_maps, idx_e = prepare(x, gate_w, w1, w3, w2)
    res = run_bass_kernel_spmd(nc, in_maps, list(range(NCORES)))
    return _combine(res, idx_e)


# revision 19
# speedup vs baseline: 1.1092x; 1.0038x over previous
"""Grouped MoE (top-2 of 8 experts, SwiGLU) on 8 Trainium2 NeuronCores.

Sharding: expert-parallel with real token dispatch. The gate (softmax +
top-2 + renormalize) is computed on host as part of the sharding step;
tokens are gathered per expert into fixed-capacity buffers (CAP = max
expert count rounded up to 128). Core c owns expert c and runs the three
SwiGLU GEMMs in bf16 over only its own ~T*K/E tokens, scales each output
row by that token's gate weight on-device, and writes its [CAP, D] f32
shard. The host scatter-adds the two expert contributions per token back
into the full [T, D] output. No collectives are needed: each token's two
expert rows live on different cores and are summed on host.
"""

import sys
import numpy as np

for _p in ("/opt/trn_rl_repo",):
    if _p not in sys.path:
        sys.path.insert(0, _p)

B, S, D, F, E, K = 2, 2048, 1024, 1024, 8, 2
T = B * S            # 4096 tokens
NCORES = 8
P = 128
DK = D // P          # 8 contraction chunks over D
FK = F // P          # 8 F tiles
MAXCH = 512          # max token chunk (PSUM bank limit: 512 f32/partition)

_cache = {}


def _build_nc(cap):
    from contextlib import ExitStack

    import concourse.mybir as mybir
    import concourse.tile as tile
    from concourse import bacc

    dt = mybir.dt
    AF = mybir.ActivationFunctionType
    ALU = mybir.AluOpType

    ntiles = (cap + P - 1) // P
    # token chunks of up to 512 f32 (PSUM bank limit).  All chunks are
    # multiples of 128 except possibly the last; avoid chunks < 128 (the
    # ~60-cycle NX dispatch floor makes n=64 matmuls cost nearly as much as
    # n=192 ones) by carving a 192 remainder when cap % 128 == 64.
    sizes = []
    rem = cap
    while rem > 512:
        if rem % 128 == 64 and rem <= 512 + 192:
            break
        sizes.append(512)
        rem -= 512
    if rem % 128 == 64 and rem > 192:
        sizes += [rem - 192, 192]
    elif rem:
        sizes.append(rem)
    chunks = []
    off = 0
    for tch in sizes:
        chunks.append((off, tch))
        off += tch

    nc = bacc.Bacc("TRN2", target_bir_lowering=False, debug=False,
                   num_devices=NCORES)

    xg = nc.dram_tensor("xg", [D, cap], dt.bfloat16, kind="ExternalInput").ap()
    gsc = nc.dram_tensor("gsc", [P, ntiles], dt.float32,
                         kind="ExternalInput").ap()
    w1t = nc.dram_tensor("w1t", [D, F], dt.bfloat16, kind="ExternalInput").ap()
    w3t = nc.dram_tensor("w3t", [D, F], dt.bfloat16, kind="ExternalInput").ap()
    w2t = nc.dram_tensor("w2t", [F, D], dt.bfloat16, kind="ExternalInput").ap()
    out = nc.dram_tensor("out", [cap, D], dt.bfloat16,
                         kind="ExternalOutput").ap()

    with tile.TileContext(nc) as tc, ExitStack() as ctx:
        const = ctx.enter_context(tc.tile_pool(name="const", bufs=1))
        xpool = ctx.enter_context(tc.tile_pool(name="xpool", bufs=1))
        spool = ctx.enter_context(tc.tile_pool(name="spool", bufs=2))
        hpool = ctx.enter_context(tc.tile_pool(name="hpool", bufs=2))
        ypool = ctx.enter_context(tc.tile_pool(name="ypool", bufs=3))

        abpsum = ctx.enter_context(tc.tile_pool(name="abpsum", bufs=2,
                                                space="PSUM"))
        ypsum = ctx.enter_context(tc.tile_pool(name="ypsum", bufs=2,
                                               space="PSUM"))

        # ---- resident loads.  Sync ring carries the stage-A critical path
        # (x chunk 0 as one multi-AP DMA, then w1 per-k) plus the x tail and
        # output stores; the scalar ring carries w3/w2 — 9 descriptor issues
        # that finish well before the first silu needs the scalar engine. ----
        xall = xpool.tile([P, DK * cap], dt.bfloat16, tag="xall")
        xg_sb = [xall[:, k * cap:(k + 1) * cap] for k in range(DK)]
        t0, tch0 = chunks[0]
        nc.sync.dma_start(
            xall[:].rearrange("p (k t) -> p k t", k=DK)[:, :, t0:tch0],
            xg.rearrange("(k p) t -> p k t", p=P)[:, :, t0:tch0])

        # w1/w3 arrive in column halves: the low halves (2 MB, serving F-tiles
        # 0..3) are the only weight bytes on the startup critical path; the
        # high halves stream in while f=0..3 compute.
        w1_sb = [const.tile([P, F], dt.bfloat16, tag=f"w1_{k}",
                            name=f"w1_{k}") for k in range(DK)]
        w3_sb = [const.tile([P, F], dt.bfloat16, tag=f"w3_{k}",
                            name=f"w3_{k}") for k in range(DK)]
        for k in range(DK):
            nc.sync.dma_start(w1_sb[k][:, 0:F // 2],
                              w1t[k * P:(k + 1) * P, 0:F // 2])
        for k in range(DK):
            nc.sync.dma_start(w3_sb[k][:, 0:F // 2],
                              w3t[k * P:(k + 1) * P, 0:F // 2])
        for k in range(DK):
            nc.sync.dma_start(w1_sb[k][:, F // 2:F],
                              w1t[k * P:(k + 1) * P, F // 2:F])
        for k in range(DK):
            nc.sync.dma_start(w3_sb[k][:, F // 2:F],
                              w3t[k * P:(k + 1) * P, F // 2:F])

        gsc_sb = const.tile([P, ntiles], dt.float32, tag="gsc")
        nc.sync.dma_start(gsc_sb[:], gsc[:, :])

        w2all = const.tile([P, FK * D], dt.bfloat16, tag="w2all")
        nc.sync.dma_start(
            w2all[:].rearrange("p (k d) -> p k d", k=FK),
            w2t.rearrange("(k p) d -> p k d", p=P))
        w2_sb = [w2all[:, k * D:(k + 1) * D] for k in range(FK)]

        if cap > tch0:
            nc.sync.dma_start(
                xall[:].rearrange("p (k t) -> p k t", k=DK)[:, :, tch0:cap],
                xg.rearrange("(k p) t -> p k t", p=P)[:, :, tch0:cap])

        # ---- PE warm-up: dummy matmuls while the weight DMAs are in flight
        # keep the tensor engine's activity window full so HAM reaches the
        # 2.4 GHz p-state before the real stream begins ----
        wrm = spool.tile([P, 512], dt.bfloat16, tag="wrm")
        nc.vector.memset(wrm[:], 0.5)
        psW = abpsum.tile([P, 512], dt.float32, tag="psA")
        for _ in range(10):
            nc.tensor.matmul(psW[:], lhsT=wrm[:, 0:P], rhs=wrm[:],
                             start=True, stop=True)

        # ---- per-chunk SwiGLU FFN ----
        for (tok, tch) in chunks:
            h_sb = []
            for f in range(FK):
                psA = abpsum.tile([P, tch], dt.float32, tag="psA")
                for k in range(DK):
                    nc.tensor.matmul(
                        psA[:], lhsT=w1_sb[k][:, f * P:(f + 1) * P],
                        rhs=xg_sb[k][:, tok:tok + tch],
                        start=(k == 0), stop=(k == DK - 1))
                psB = abpsum.tile([P, tch], dt.float32, tag="psB")
                for k in range(DK):
                    nc.tensor.matmul(
                        psB[:], lhsT=w3_sb[k][:, f * P:(f + 1) * P],
                        rhs=xg_sb[k][:, tok:tok + tch],
                        start=(k == 0), stop=(k == DK - 1))
                ssb = spool.tile([P, tch], dt.bfloat16, tag="ssb")
                nc.scalar.activation(ssb[:], psA[:], AF.Silu)
                hsb = hpool.tile([P, tch], dt.bfloat16, tag=f"h{f}")
                nc.vector.tensor_tensor(hsb[:], ssb[:], psB[:], op=ALU.mult)
                h_sb.append(hsb)
            for m in range((tch + P - 1) // P):
                jj = tok // P + m
                pm = min(P, tch - m * P)
                last = (tok + m * P + pm == cap)
                psY = ypsum.tile([P, D], dt.float32, tag="psY")
                for nhalf in range(2):
                    for fk in range(FK):
                        nc.tensor.matmul(
                            psY[:pm, nhalf * 512:(nhalf + 1) * 512],
                            lhsT=h_sb[fk][:, m * P:m * P + pm],
                            rhs=w2_sb[fk][:, nhalf * 512:(nhalf + 1) * 512],
                            start=(fk == 0), stop=(fk == FK - 1))
                ysb = ypool.tile([P, D], dt.bfloat16, tag="ysb")
                if last:
                    # split the final copy+store so the first half's DMA
                    # overlaps the second half's copy (shorter kernel tail)
                    for nhalf in range(2):
                        nc.scalar.activation(
                            ysb[:pm, nhalf * 512:(nhalf + 1) * 512],
                            psY[:pm, nhalf * 512:(nhalf + 1) * 512],
                            AF.Copy, scale=gsc_sb[:pm, jj:jj + 1])
                        nc.sync.dma_start(
                            out[jj * P:jj * P + pm,
                                nhalf * 512:(nhalf + 1) * 512],
                            ysb[:pm, nhalf * 512:(nhalf + 1) * 512])
                else:
                    nc.scalar.activation(ysb[:pm, :], psY[:pm, :], AF.Copy,
                                         scale=gsc_sb[:pm, jj:jj + 1])
                    nc.sync.dma_start(out[jj * P:jj * P + pm, :], ysb[:pm, :])

    nc.compile()
    return nc


def _route(xf, gate_w):
    """Host gate: returns per-expert (token indices, renormalized weights)."""
    logits = xf.astype(np.float64) @ gate_w.astype(np.float64).T   # [T, E]
    order = np.argsort(-logits, axis=1, kind="stable")
    i1 = order[:, 0]
    i2 = order[:, 1]
    ar = np.arange(T)
    l1 = logits[ar, i1]
    l2 = logits[ar, i2]
    g1 = 1.0 / (1.0 + np.exp(l2 - l1))
    g2 = 1.0 - g1
    idx_e, scl_e = [], []
    for e in range(E):
        m1 = i1 == e
        m2 = i2 == e
        ids = np.concatenate([np.nonzero(m1)[0], np.nonzero(m2)[0]])
        sc = np.concatenate([g1[m1], g2[m2]])
        idx_e.append(ids)
        scl_e.append(sc.astype(np.float32))
    return idx_e, scl_e


def prepare(x, gate_w, w1, w3, w2):
    """Host routing + sharding: returns (nc, in_maps, idx_e)."""
    import ml_dtypes

    xf = np.ascontiguousarray(x.reshape(T, D).astype(np.float32))
    xTb = np.ascontiguousarray(xf.T).astype(ml_dtypes.bfloat16)   # [D, T]

    idx_e, scl_e = _route(xf, gate_w)
    maxcnt = max(len(i) for i in idx_e)
    cap = ((maxcnt + 63) // 64) * 64
    ntiles = (cap + P - 1) // P

    if cap not in _cache:
        _cache[cap] = _build_nc(cap)
    nc = _cache[cap]

    in_maps = []
    for c in range(NCORES):
        ids = idx_e[c]
        cnt = len(ids)
        xg_c = np.zeros((D, cap), dtype=ml_dtypes.bfloat16)
        xg_c[:, :cnt] = xTb[:, ids]
        sc = np.zeros(ntiles * P, dtype=np.float32)
        sc[:cnt] = scl_e[c]
        gsc_c = np.ascontiguousarray(sc.reshape(ntiles, P).T)     # [P, ntiles]
        in_maps.append({
            "xg": xg_c,
            "gsc": gsc_c,
            "w1t": np.ascontiguousarray(w1[c].T).astype(ml_dtypes.bfloat16),
            "w3t": np.ascontiguousarray(w3[c].T).astype(ml_dtypes.bfloat16),
            "w2t": np.ascontiguousarray(w2[c].T).astype(ml_dtypes.bfloat16),
        })
    return nc, in_maps, idx_e


def _combine(res, idx_e):
    outf = np.zeros((T, D), dtype=np.float32)
    for c in range(NCORES):
        cnt = len(idx_e[c])
        outf[idx_e[c]] += res.results[c]["out"][:cnt].astype(np.float32)
    return outf.reshape(B, S, D)


def kernel(x, gate_w, w1, w3, w2):
    from concourse.bass_utils import run_bass_kernel_spmd

    nc, in_maps, idx_e = prepare(x, gate_w, w1, w3, w2)
    res = run_bass_kernel_spmd(nc, in_maps, list(range(NCORES)))
    return _combine(res, idx_e)


# revision 20
# speedup vs baseline: 1.1304x; 1.0191x over previous
"""Grouped MoE (top-2 of 8 experts, SwiGLU) on 8 Trainium2 NeuronCores.

Sharding: expert-parallel with real token dispatch. The gate (softmax +
top-2 + renormalize) is computed on host as part of the sharding step;
tokens are gathered per expert into fixed-capacity buffers (CAP = max
expert count rounded up to 128). Core c owns expert c and runs the three
SwiGLU GEMMs in bf16 over only its own ~T*K/E tokens, scales each output
row by that token's gate weight on-device, and writes its [CAP, D] f32
shard. The host scatter-adds the two expert contributions per token back
into the full [T, D] output. No collectives are needed: each token's two
expert rows live on different cores and are summed on host.
"""

import sys
import numpy as np

for _p in ("/opt/trn_rl_repo",):
    if _p not in sys.path:
        sys.path.insert(0, _p)

B, S, D, F, E, K = 2, 2048, 1024, 1024, 8, 2
T = B * S            # 4096 tokens
NCORES = 8
P = 128
DK = D // P          # 8 contraction chunks over D
FK = F // P          # 8 F tiles
MAXCH = 512          # max token chunk (PSUM bank limit: 512 f32/partition)

_cache = {}


def _build_nc(cap):
    from contextlib import ExitStack

    import concourse.mybir as mybir
    import concourse.tile as tile
    from concourse import bacc

    dt = mybir.dt
    AF = mybir.ActivationFunctionType
    ALU = mybir.AluOpType

    ntiles = (cap + P - 1) // P
    # token chunks of up to 512 f32 (PSUM bank limit).  All chunks are
    # multiples of 128 except possibly the last; avoid chunks < 128 (the
    # ~60-cycle NX dispatch floor makes n=64 matmuls cost nearly as much as
    # n=192 ones) by carving a 192 remainder when cap % 128 == 64.
    sizes = []
    rem = cap
    while rem > 512:
        if rem % 128 == 64 and rem <= 512 + 192:
            break
        sizes.append(512)
        rem -= 512
    if rem % 128 == 64 and rem > 192:
        sizes += [rem - 192, 192]
    elif rem:
        sizes.append(rem)
    chunks = []
    off = 0
    for tch in sizes:
        chunks.append((off, tch))
        off += tch

    nc = bacc.Bacc("TRN2", target_bir_lowering=False, debug=False,
                   num_devices=NCORES)

    xg = nc.dram_tensor("xg", [D, cap], dt.bfloat16, kind="ExternalInput").ap()
    gsc = nc.dram_tensor("gsc", [P, ntiles], dt.float32,
                         kind="ExternalInput").ap()
    w1t = nc.dram_tensor("w1t", [D, F], dt.bfloat16, kind="ExternalInput").ap()
    w3t = nc.dram_tensor("w3t", [D, F], dt.bfloat16, kind="ExternalInput").ap()
    w2t = nc.dram_tensor("w2t", [F, D], dt.bfloat16, kind="ExternalInput").ap()
    out = nc.dram_tensor("out", [cap, D], dt.bfloat16,
                         kind="ExternalOutput").ap()

    with tile.TileContext(nc) as tc, ExitStack() as ctx:
        const = ctx.enter_context(tc.tile_pool(name="const", bufs=1))
        xpool = ctx.enter_context(tc.tile_pool(name="xpool", bufs=1))
        spool = ctx.enter_context(tc.tile_pool(name="spool", bufs=2))
        hpool = ctx.enter_context(tc.tile_pool(name="hpool", bufs=2))
        ypool = ctx.enter_context(tc.tile_pool(name="ypool", bufs=3))

        abpsum = ctx.enter_context(tc.tile_pool(name="abpsum", bufs=2,
                                                space="PSUM"))
        ypsum = ctx.enter_context(tc.tile_pool(name="ypsum", bufs=2,
                                               space="PSUM"))

        # ---- resident loads.  Sync ring carries the stage-A critical path
        # (x chunk 0 as one multi-AP DMA, then w1 per-k) plus the x tail and
        # output stores; the scalar ring carries w3/w2 — 9 descriptor issues
        # that finish well before the first silu needs the scalar engine. ----
        xall = xpool.tile([P, DK * cap], dt.bfloat16, tag="xall")
        xg_sb = [xall[:, k * cap:(k + 1) * cap] for k in range(DK)]
        t0, tch0 = chunks[0]
        nc.sync.dma_start(
            xall[:].rearrange("p (k t) -> p k t", k=DK)[:, :, t0:tch0],
            xg.rearrange("(k p) t -> p k t", p=P)[:, :, t0:tch0])

        # w1/w3 arrive in column halves: the low halves (2 MB, serving F-tiles
        # 0..3) are the only weight bytes on the startup critical path; the
        # high halves stream in while f=0..3 compute.
        w1_sb = [const.tile([P, F], dt.bfloat16, tag=f"w1_{k}",
                            name=f"w1_{k}") for k in range(DK)]
        w3_sb = [const.tile([P, F], dt.bfloat16, tag=f"w3_{k}",
                            name=f"w3_{k}") for k in range(DK)]
        for k in range(DK):
            nc.sync.dma_start(w1_sb[k][:, 0:F // 2],
                              w1t[k * P:(k + 1) * P, 0:F // 2])
        for k in range(DK):
            nc.sync.dma_start(w3_sb[k][:, 0:F // 2],
                              w3t[k * P:(k + 1) * P, 0:F // 2])
        for k in range(DK):
            nc.sync.dma_start(w1_sb[k][:, F // 2:F],
                              w1t[k * P:(k + 1) * P, F // 2:F])
        for k in range(DK):
            nc.sync.dma_start(w3_sb[k][:, F // 2:F],
                              w3t[k * P:(k + 1) * P, F // 2:F])

        gsc_sb = const.tile([P, ntiles], dt.float32, tag="gsc")
        nc.sync.dma_start(gsc_sb[:], gsc[:, :])

        w2all = const.tile([P, FK * D], dt.bfloat16, tag="w2all")
        nc.sync.dma_start(
            w2all[:].rearrange("p (k d) -> p k d", k=FK),
            w2t.rearrange("(k p) d -> p k d", p=P))
        w2_sb = [w2all[:, k * D:(k + 1) * D] for k in range(FK)]

        if cap > tch0:
            nc.sync.dma_start(
                xall[:].rearrange("p (k t) -> p k t", k=DK)[:, :, tch0:cap],
                xg.rearrange("(k p) t -> p k t", p=P)[:, :, tch0:cap])

        # ---- PE warm-up: dummy matmuls while the weight DMAs are in flight
        # keep the tensor engine's activity window full so HAM reaches the
        # 2.4 GHz p-state before the real stream begins ----
        wrm = spool.tile([P, 512], dt.bfloat16, tag="wrm")
        nc.vector.memset(wrm[:], 0.5)
        psW = abpsum.tile([P, 512], dt.float32, tag="psA")
        for _ in range(16):
            nc.tensor.matmul(psW[:], lhsT=wrm[:, 0:P], rhs=wrm[:],
                             start=True, stop=True)

        # ---- per-chunk SwiGLU FFN ----
        for (tok, tch) in chunks:
            h_sb = []
            for f in range(FK):
                psA = abpsum.tile([P, tch], dt.float32, tag="psA")
                for k in range(DK):
                    nc.tensor.matmul(
                        psA[:], lhsT=w1_sb[k][:, f * P:(f + 1) * P],
                        rhs=xg_sb[k][:, tok:tok + tch],
                        start=(k == 0), stop=(k == DK - 1))
                psB = abpsum.tile([P, tch], dt.float32, tag="psB")
                for k in range(DK):
                    nc.tensor.matmul(
                        psB[:], lhsT=w3_sb[k][:, f * P:(f + 1) * P],
                        rhs=xg_sb[k][:, tok:tok + tch],
                        start=(k == 0), stop=(k == DK - 1))
                ssb = spool.tile([P, tch], dt.bfloat16, tag="ssb")
                nc.scalar.activation(ssb[:], psA[:], AF.Silu)
                hsb = hpool.tile([P, tch], dt.bfloat16, tag=f"h{f}")
                nc.vector.tensor_tensor(hsb[:], ssb[:], psB[:], op=ALU.mult)
                h_sb.append(hsb)
            for m in range((tch + P - 1) // P):
                jj = tok // P + m
                pm = min(P, tch - m * P)
                last = (tok + m * P + pm == cap)
                psY = ypsum.tile([P, D], dt.float32, tag="psY")
                for nhalf in range(2):
                    for fk in range(FK):
                        nc.tensor.matmul(
                            psY[:pm, nhalf * 512:(nhalf + 1) * 512],
                            lhsT=h_sb[fk][:, m * P:m * P + pm],
                            rhs=w2_sb[fk][:, nhalf * 512:(nhalf + 1) * 512],
                            start=(fk == 0), stop=(fk == FK - 1))
                ysb = ypool.tile([P, D], dt.bfloat16, tag="ysb")
                if last:
                    # split the final copy+store so the first half's DMA
                    # overlaps the second half's copy (shorter kernel tail)
                    for nhalf in range(2):
                        nc.scalar.activation(
                            ysb[:pm, nhalf * 512:(nhalf + 1) * 512],
                            psY[:pm, nhalf * 512:(nhalf + 1) * 512],
                            AF.Copy, scale=gsc_sb[:pm, jj:jj + 1])
                        nc.sync.dma_start(
                            out[jj * P:jj * P + pm,
                                nhalf * 512:(nhalf + 1) * 512],
                            ysb[:pm, nhalf * 512:(nhalf + 1) * 512])
                else:
                    nc.scalar.activation(ysb[:pm, :], psY[:pm, :], AF.Copy,
                                         scale=gsc_sb[:pm, jj:jj + 1])
                    nc.sync.dma_start(out[jj * P:jj * P + pm, :], ysb[:pm, :])

    nc.compile()
    return nc


def _route(xf, gate_w):
    """Host gate: returns per-expert (token indices, renormalized weights)."""
    logits = xf.astype(np.float64) @ gate_w.astype(np.float64).T   # [T, E]
    order = np.argsort(-logits, axis=1, kind="stable")
    i1 = order[:, 0]
    i2 = order[:, 1]
    ar = np.arange(T)
    l1 = logits[ar, i1]
    l2 = logits[ar, i2]
    g1 = 1.0 / (1.0 + np.exp(l2 - l1))
    g2 = 1.0 - g1
    idx_e, scl_e = [], []
    for e in range(E):
        m1 = i1 == e
        m2 = i2 == e
        ids = np.concatenate([np.nonzero(m1)[0], np.nonzero(m2)[0]])
        sc = np.concatenate([g1[m1], g2[m2]])
        idx_e.append(ids)
        scl_e.append(sc.astype(np.float32))
    return idx_e, scl_e


def prepare(x, gate_w, w1, w3, w2):
    """Host routing + sharding: returns (nc, in_maps, idx_e)."""
    import ml_dtypes

    xf = np.ascontiguousarray(x.reshape(T, D).astype(np.float32))
    xTb = np.ascontiguousarray(xf.T).astype(ml_dtypes.bfloat16)   # [D, T]

    idx_e, scl_e = _route(xf, gate_w)
    maxcnt = max(len(i) for i in idx_e)
    cap = ((maxcnt + 63) // 64) * 64
    ntiles = (cap + P - 1) // P

    if cap not in _cache:
        _cache[cap] = _build_nc(cap)
    nc = _cache[cap]

    in_maps = []
    for c in range(NCORES):
        ids = idx_e[c]
        cnt = len(ids)
        xg_c = np.zeros((D, cap), dtype=ml_dtypes.bfloat16)
        xg_c[:, :cnt] = xTb[:, ids]
        sc = np.zeros(ntiles * P, dtype=np.float32)
        sc[:cnt] = scl_e[c]
        gsc_c = np.ascontiguousarray(sc.reshape(ntiles, P).T)     # [P, ntiles]
        in_maps.append({
            "xg": xg_c,
            "gsc": gsc_c,
            "w1t": np.ascontiguousarray(w1[c].T).astype(ml_dtypes.bfloat16),
            "w3t": np.ascontiguousarray(w3[c].T).astype(ml_dtypes.bfloat16),
            "w2t": np.ascontiguousarray(w2[c].T).astype(ml_dtypes.bfloat16),
        })
    return nc, in_maps, idx_e


def _combine(res, idx_e):
    outf = np.zeros((T, D), dtype=np.float32)
    for c in range(NCORES):
        cnt = len(idx_e[c])
        outf[idx_e[c]] += res.results[c]["out"][:cnt].astype(np.float32)
    return outf.reshape(B, S, D)


def kernel(x, gate_w, w1, w3, w2):
    from concourse.bass_utils import run_bass_kernel_spmd

    nc, in_maps, idx_e = prepare(x, gate_w, w1, w3, w2)
    res = run_bass_kernel_spmd(nc, in_maps, list(range(NCORES)))
    return _combine(res, idx_e)
